# revision 11
# baseline (speedup 1.0000x reference)
"""Additive attention (Bahdanau) Trainium2 Bass kernel.

Sharding: data-parallel over batch B=64 -> 8 cores x 8 batches.
Per core, per batch b:
  proj[d, n] = W_img @ img[b, n, :]            PE, [d,n] tiles, lhsT=W_imgT
  tanh tiles = tanh(proj + ph[d])              ACT, per-partition bias fused
  scores col [n, 1] = tanh_chunk.T @ W_score   PE, lhsT=tanh chunk (col layout)
  exp -> softmax (all state at partition 0)    ACT accum/scale + tiny PE ops
  context = sum_n w[n] * img[b, n, :]          PE, lhsT = normalized w column

TRN2 instructions embed only ONE sync wait, so every op is arranged to have
at most one unobserved cross-engine dependency (Tile elides the rest via its
per-engine vector clock).
"""

import sys
import numpy as np

for p in ("/opt/trn_rl_repo",):
    if p not in sys.path:
        sys.path.insert(0, p)

import ml_dtypes

B, N, F, H, D = 64, 4096, 512, 512, 256
NCORES = 8
BPC = B // NCORES  # batches per core
NT = 512           # n-tile size for proj phase
NNT = N // NT      # 8 n-tiles
NCH = N // 128     # 32 n-chunks

_BF = ml_dtypes.bfloat16

_nc_cache = {}


def _build_nc():
    if "nc" in _nc_cache:
        return _nc_cache["nc"]
    from contextlib import ExitStack

    import concourse.bass as bass  # noqa: F401
    import concourse.tile as tile
    from concourse import bacc, mybir

    bf16 = mybir.dt.bfloat16
    f32 = mybir.dt.float32
    AF = mybir.ActivationFunctionType

    nc = bacc.Bacc("TRN2")

    imgT = nc.declare_dram_parameter("imgT", [BPC, F, N], bf16, isOutput=False)
    imgN = nc.declare_dram_parameter("imgN", [BPC, N, F], bf16, isOutput=False)
    # hw = [W_hid.T | hidden.T] packed: one DMA -> one wait on the f32 matmul
    hw = nc.declare_dram_parameter("hw", [H, D + BPC], f32, isOutput=False)
    wimgT = nc.declare_dram_parameter("wimgT", [F, D], bf16, isOutput=False)
    wscore = nc.declare_dram_parameter("wscore", [D, 1], bf16, isOutput=False)
    ident = nc.declare_dram_parameter("ident", [128, 128], f32, isOutput=False)
    ctx_out = nc.declare_dram_parameter("ctx", [BPC, F], f32, isOutput=True)
    wts_out = nc.declare_dram_parameter("wts", [BPC, N], f32, isOutput=True)

    with tile.TileContext(nc) as tc, ExitStack() as ctx:
        const = ctx.enter_context(tc.tile_pool(name="const", bufs=1))
        itp = ctx.enter_context(tc.tile_pool(name="imgTp", bufs=3))
        inp = ctx.enter_context(tc.tile_pool(name="imgNp", bufs=4))
        thp = ctx.enter_context(tc.tile_pool(name="tanh", bufs=4))
        expp = ctx.enter_context(tc.tile_pool(name="expp", bufs=2))
        smp = ctx.enter_context(tc.tile_pool(name="smp", bufs=2))
        csp = ctx.enter_context(tc.tile_pool(name="csb", bufs=2))
        pps = ctx.enter_context(tc.tile_pool(name="pp", bufs=2, space="PSUM"))
        scp = ctx.enter_context(tc.tile_pool(name="sc", bufs=1, space="PSUM"))
        spp = ctx.enter_context(tc.tile_pool(name="sp", bufs=1, space="PSUM"))
        ctxps = ctx.enter_context(tc.tile_pool(name="cps", bufs=2, space="PSUM"))

        # ---- constants (consumers arranged for <=1 wait each) ----
        wimg_sb = const.tile([128, 4, D], bf16)
        nc.sync.dma_start(wimg_sb[:], wimgT.rearrange("(c p) d -> p c d", p=128))
        hw_sb = const.tile([128, 4, D + BPC], f32)
        nc.sync.dma_start(hw_sb[:], hw.rearrange("(c p) x -> p c x", p=128))
        wsc_dma = const.tile([128, 2, 1], bf16)
        nc.sync.dma_start(wsc_dma[:], wscore.rearrange("(c p) o -> p c o", p=128))
        wsc_sb = const.tile([128, 2, 1], bf16)
        nc.scalar.copy(wsc_sb[:], wsc_dma[:])
        id_dma = const.tile([128, 128], f32)
        nc.sync.dma_start(id_dma[:], ident[:])
        id_sb = const.tile([128, 128], f32)
        nc.scalar.copy(id_sb[:], id_dma[:])
        ones_col = const.tile([128, 1], f32)
        nc.vector.memset(ones_col[:], 1.0)
        ones_row = const.tile([1, 128], f32)
        nc.vector.memset(ones_row[:], 1.0)
        exp_scr = const.tile([128, NCH], f32)  # scratch dest for accum pass

        # PE observes the wimg DMA lane once, so proj matmuls carry only
        # their own img-tile wait afterwards.
        obs_ps = spp.tile([1, 1], f32, tag="sp")
        nc.tensor.matmul(obs_ps[:], lhsT=wimg_sb[:, 0, 0:1],
                         rhs=wimg_sb[:, 0, 0:1], start=True, stop=True)

        # ---- proj_hidden in [d, b] layout (per-partition bias for tanh) ----
        ph_ps = spp.tile([128, 2, BPC], f32, tag="sp")
        for dh in range(2):
            for hc in range(4):
                nc.tensor.matmul(
                    ph_ps[:, dh, :],
                    lhsT=hw_sb[:, hc, dh * 128:(dh + 1) * 128],
                    rhs=hw_sb[:, hc, D:D + BPC],
                    start=(hc == 0),
                    stop=(hc == 3),
                )
        ph_sb = const.tile([128, 2, BPC], f32)
        nc.scalar.copy(ph_sb[:], ph_ps[:])

        for b in range(BPC):
            # ---- phase A: proj + tanh + scores-col + exp ----
            expT = expp.tile([128, NCH], f32, tag="expT")  # [n%128, n//128]
            for nt in range(NNT):
                it = itp.tile([128, 4, NT], bf16, tag="it")
                nc.sync.dma_start(
                    it[:],
                    imgT[b].rearrange("(c p) n -> p c n", p=128)[
                        :, :, nt * NT:(nt + 1) * NT
                    ],
                )
                pp = pps.tile([128, 2, NT], f32, tag="pp")  # 2 PSUM banks
                ths = []
                for dh in range(2):
                    for fc in range(4):
                        nc.tensor.matmul(
                            pp[:, dh, :],
                            lhsT=wimg_sb[:, fc, dh * 128:(dh + 1) * 128],
                            rhs=it[:, fc, :],
                            start=(fc == 0),
                            stop=(fc == 3),
                        )
                    th = thp.tile([128, NT], bf16, tag="th")
                    nc.scalar.activation(
                        th[:], pp[:, dh, :], AF.Tanh,
                        bias=ph_sb[:, dh, b:b + 1], scale=1.0,
                    )
                    ths.append(th)
                sc = scp.tile([128, 4], f32, tag="sc")
                for ns in range(4):
                    for dh in range(2):
                        nc.tensor.matmul(
                            sc[:, ns:ns + 1],
                            lhsT=ths[dh][:, ns * 128:(ns + 1) * 128],
                            rhs=wsc_sb[:, dh, :],
                            start=(dh == 0),
                            stop=(dh == 1),
                        )
                nc.scalar.activation(
                    expT[:, nt * 4:(nt + 1) * 4], sc[:], AF.Exp
                )

            # ---- phase B: softmax normalize (ACT-centric, 1-wait shapes) ----
            sexp = smp.tile([128, 1], f32, tag="sexp")
            nc.scalar.activation(exp_scr[:], expT[:], AF.Copy, accum_out=sexp[:])
            ssum_ps = spp.tile([1, 1], f32, tag="sp")
            nc.tensor.matmul(ssum_ps[:], lhsT=sexp[:], rhs=ones_col[:],
                             start=True, stop=True)
            rec = smp.tile([1, 1], f32, tag="rec")
            nc.vector.reciprocal(rec[:], ssum_ps[:])
            rec_ps = spp.tile([128, 1], f32, tag="sp")
            nc.tensor.matmul(rec_ps[:], lhsT=ones_row[:], rhs=rec[:],
                             start=True, stop=True)
            rec_bc = smp.tile([128, 1], f32, tag="recbc")
            nc.scalar.copy(rec_bc[:], rec_ps[:])
            wn_bf = smp.tile([128, NCH], bf16, tag="wnbf")
            nc.scalar.activation(wn_bf[:], expT[:], AF.Copy, scale=rec_bc[:])
            wn_f = smp.tile([128, NCH], f32, tag="wnf")
            nc.scalar.activation(wn_f[:], expT[:], AF.Copy, scale=rec_bc[:])
            # weights out: transpose [128, 32] -> [32, 128], contiguous DMA
            wtr_ps = spp.tile([NCH, 128], f32, tag="sp")
            nc.tensor.transpose(wtr_ps[:], wn_f[:], id_sb[:])
            wtr_sb = smp.tile([NCH, 128], f32, tag="wtr")
            nc.vector.tensor_copy(wtr_sb[:], wtr_ps[:])
            nc.sync.dma_start(
                wts_out[b].rearrange("(c p) -> c p", p=128), wtr_sb[:]
            )

            # ---- phase C: context ----
            cp = ctxps.tile([1, F], f32, tag="cp")
            for c in range(NCH):
                inat = inp.tile([128, F], bf16, tag="in")
                nc.sync.dma_start(inat[:], imgN[b, c * 128:(c + 1) * 128, :])
                nc.tensor.matmul(
                    cp[:],
                    lhsT=wn_bf[:, c:c + 1],
                    rhs=inat[:],
                    start=(c == 0),
                    stop=(c == NCH - 1),
                )
            csb = csp.tile([1, F], f32, tag="csb")
            nc.vector.tensor_copy(csb[:], cp[:])
            nc.sync.dma_start(ctx_out[b:b + 1, :], csb[:])

    nc.compile()
    _nc_cache["nc"] = nc
    return nc


def _in_maps(image_features, hidden_state, W_img, W_hid, W_score):
    img_bf = image_features.astype(_BF)                        # [64, 4096, 512]
    imgT_bf = np.ascontiguousarray(img_bf.transpose(0, 2, 1))  # [64, 512, 4096]
    wimgT = np.ascontiguousarray(W_img.T).astype(_BF)          # [512, 256]
    whidT = W_hid.T.astype(np.float32)                         # [512, 256]
    wsc = np.ascontiguousarray(W_score.reshape(1, D).T).astype(_BF)  # [256, 1]
    eye = np.eye(128, dtype=np.float32)

    in_maps = []
    for c in range(NCORES):
        s = slice(c * BPC, (c + 1) * BPC)
        hwpack = np.concatenate(
            [whidT, hidden_state[s].T.astype(np.float32)], axis=1
        )  # [512, 264]
        in_maps.append({
            "imgT": imgT_bf[s],
            "imgN": img_bf[s],
            "hw": np.ascontiguousarray(hwpack),
            "wimgT": wimgT,
            "wscore": wsc,
            "ident": eye,
        })
    return in_maps


def kernel(image_features, hidden_state, W_img, W_hid, W_score):
    from concourse.bass_utils import run_bass_kernel_spmd

    nc = _build_nc()
    in_maps = _in_maps(image_features, hidden_state, W_img, W_hid, W_score)
    res = run_bass_kernel_spmd(nc, in_maps, list(range(NCORES))).results
    ctx = np.concatenate([r["ctx"] for r in res], axis=0)
    wts = np.concatenate([r["wts"] for r in res], axis=0)
    return (ctx, wts)


# revision 13
# speedup vs baseline: 1.1032x; 1.1032x over previous
"""Additive attention (Bahdanau) Trainium2 Bass kernel.

Sharding: data-parallel over batch B=64 -> 8 cores x 8 batches.
Per core, per batch b:
  proj[d, n] = W_img @ img[b, n, :]            PE, [d,n] tiles, lhsT=W_imgT
  tanh tiles = tanh(proj + ph[d])              ACT, per-partition bias fused
  scores col [n, 1] = tanh_chunk.T @ W_score   PE, lhsT=tanh chunk (col layout)
  exp -> softmax (all state at partition 0)    ACT accum/scale + tiny PE ops
  context = sum_n w[n] * img[b, n, :]          PE, lhsT = normalized w column

Host feeds partition-tiled layouts so every DMA has 4 KiB contiguous runs
per partition:
  imgT_q [8, 128, 4, 4096]  (p, f-chunk, n)  for proj rhs tiles
  imgN_q [8, 128, 32, 512]  (p, n-chunk, f)  for context rhs tiles

TRN2 instructions embed only ONE sync wait; Bacc's generate_event_semaphores
legalizes the rest. Emission order pipelines softmax(b)/context(b) against
proj(b+1) to keep PE dense (HAM warm).
"""

import sys
import numpy as np

for p in ("/opt/trn_rl_repo",):
    if p not in sys.path:
        sys.path.insert(0, p)

import ml_dtypes

B, N, F, H, D = 64, 4096, 512, 512, 256
NCORES = 8
BPC = B // NCORES  # batches per core
NT = 2048          # n-tile size for proj loads
NNT = N // NT      # 2 tiles
NSUB = NT // 512   # 4 x 512 matmul slices per tile
NCH = N // 128     # 32 n-chunks
CG = 4             # context chunks per load
_BF = ml_dtypes.bfloat16

_nc_cache = {}


def _build_nc():
    if "nc" in _nc_cache:
        return _nc_cache["nc"]
    from contextlib import ExitStack

    import concourse.bass as bass  # noqa: F401
    import concourse.tile as tile
    from concourse import bacc, mybir

    bf16 = mybir.dt.bfloat16
    f32 = mybir.dt.float32
    AF = mybir.ActivationFunctionType

    nc = bacc.Bacc("TRN2")

    imgT = nc.declare_dram_parameter("imgT", [BPC, 128, 4, N], bf16, isOutput=False)
    imgN = nc.declare_dram_parameter("imgN", [BPC, 128, NCH, F], bf16, isOutput=False)
    # hw = [W_hid.T | hidden.T] packed: one DMA -> one wait on the f32 matmul
    hw = nc.declare_dram_parameter("hw", [H, D + BPC], f32, isOutput=False)
    wimgT = nc.declare_dram_parameter("wimgT", [F, D], bf16, isOutput=False)
    wscore = nc.declare_dram_parameter("wscore", [D, 1], bf16, isOutput=False)
    ident = nc.declare_dram_parameter("ident", [128, 128], f32, isOutput=False)
    ctx_out = nc.declare_dram_parameter("ctx", [BPC, F], f32, isOutput=True)
    wts_out = nc.declare_dram_parameter("wts", [BPC, N], f32, isOutput=True)

    with tile.TileContext(nc) as tc, ExitStack() as ctx:
        const = ctx.enter_context(tc.tile_pool(name="const", bufs=1))
        itp = ctx.enter_context(tc.tile_pool(name="imgTp", bufs=3))
        inp = ctx.enter_context(tc.tile_pool(name="imgNp", bufs=3))
        thp = ctx.enter_context(tc.tile_pool(name="tanh", bufs=4))
        expp = ctx.enter_context(tc.tile_pool(name="expp", bufs=2))
        smp = ctx.enter_context(tc.tile_pool(name="smp", bufs=2))
        csp = ctx.enter_context(tc.tile_pool(name="csb", bufs=2))
        pps = ctx.enter_context(tc.tile_pool(name="pp", bufs=2, space="PSUM"))
        scp = ctx.enter_context(tc.tile_pool(name="sc", bufs=1, space="PSUM"))
        spp = ctx.enter_context(tc.tile_pool(name="sp", bufs=1, space="PSUM"))
        ctxps = ctx.enter_context(tc.tile_pool(name="cps", bufs=1, space="PSUM"))

        # ---- constants (consumers arranged for <=1 wait each) ----
        wimg_sb = const.tile([128, 4, D], bf16)
        nc.sync.dma_start(wimg_sb[:], wimgT.rearrange("(c p) d -> p c d", p=128))
        hw_sb = const.tile([128, 4, D + BPC], f32)
        nc.sync.dma_start(hw_sb[:], hw.rearrange("(c p) x -> p c x", p=128))
        wsc_dma = const.tile([128, 2, 1], bf16)
        nc.sync.dma_start(wsc_dma[:], wscore.rearrange("(c p) o -> p c o", p=128))
        wsc_sb = const.tile([128, 2, 1], bf16)
        nc.scalar.copy(wsc_sb[:], wsc_dma[:])
        id_dma = const.tile([128, 128], f32)
        nc.sync.dma_start(id_dma[:], ident[:])
        id_sb = const.tile([128, 128], f32)
        nc.scalar.copy(id_sb[:], id_dma[:])
        ones_col = const.tile([128, 1], f32)
        nc.vector.memset(ones_col[:], 1.0)
        ones_row = const.tile([1, 128], f32)
        nc.vector.memset(ones_row[:], 1.0)
        exp_scr = const.tile([128, NCH], f32)  # scratch dest for accum pass

        # PE observes the wimg DMA lane once, so proj matmuls carry only
        # their own img-tile wait afterwards.
        obs_ps = spp.tile([1, 1], f32, tag="obs")
        nc.tensor.matmul(obs_ps[:], lhsT=wimg_sb[:, 0, 0:1],
                         rhs=wimg_sb[:, 0, 0:1], start=True, stop=True)

        # ---- proj_hidden in [d, b] layout (per-partition bias for tanh) ----
        ph_ps = spp.tile([128, 2, BPC], f32, tag="sp")
        for dh in range(2):
            for hc in range(4):
                nc.tensor.matmul(
                    ph_ps[:, dh, :],
                    lhsT=hw_sb[:, hc, dh * 128:(dh + 1) * 128],
                    rhs=hw_sb[:, hc, D:D + BPC],
                    start=(hc == 0),
                    stop=(hc == 3),
                )
        ph_sb = const.tile([128, 2, BPC], f32)
        nc.scalar.copy(ph_sb[:], ph_ps[:])

        exp_tiles = {}

        def phase_a(b):
            """proj + tanh + scores-col + exp for one batch."""
            expT = expp.tile([128, NCH], f32, tag="expT")  # [n%128, n//128]
            exp_tiles[b] = expT
            for nt in range(NNT):
                it = itp.tile([128, 4, NT], bf16, tag="it")
                nc.sync.dma_start(it[:], imgT[b, :, :, nt * NT:(nt + 1) * NT])
                for sub in range(NSUB):
                    sl = slice(sub * 512, (sub + 1) * 512)
                    pp = pps.tile([128, 2, 512], f32, tag="pp")
                    ths = []
                    for dh in range(2):
                        for fc in range(4):
                            nc.tensor.matmul(
                                pp[:, dh, :],
                                lhsT=wimg_sb[:, fc, dh * 128:(dh + 1) * 128],
                                rhs=it[:, fc, sl],
                                start=(fc == 0),
                                stop=(fc == 3),
                            )
                        th = thp.tile([128, 512], bf16, tag="th")
                        nc.scalar.activation(
                            th[:], pp[:, dh, :], AF.Tanh,
                            bias=ph_sb[:, dh, b:b + 1], scale=1.0,
                        )
                        ths.append(th)
                    sc = scp.tile([128, 4], f32, tag="sc")
                    for ns in range(4):
                        for dh in range(2):
                            nc.tensor.matmul(
                                sc[:, ns:ns + 1],
                                lhsT=ths[dh][:, ns * 128:(ns + 1) * 128],
                                rhs=wsc_sb[:, dh, :],
                                start=(dh == 0),
                                stop=(dh == 1),
                            )
                    cc = nt * (NT // 128) + sub * 4
                    nc.scalar.activation(expT[:, cc:cc + 4], sc[:], AF.Exp)

        def phase_b(b):
            """softmax normalize; ACT-centric so waits stay single."""
            expT = exp_tiles[b]
            sexp = smp.tile([128, 1], f32, tag="sexp")
            nc.scalar.activation(exp_scr[:], expT[:], AF.Copy, accum_out=sexp[:])
            ssum_ps = spp.tile([1, 1], f32, tag="sp")
            nc.tensor.matmul(ssum_ps[:], lhsT=sexp[:], rhs=ones_col[:],
                             start=True, stop=True)
            rec = smp.tile([1, 1], f32, tag="rec")
            nc.vector.reciprocal(rec[:], ssum_ps[:])
            rec_ps = spp.tile([128, 1], f32, tag="sp")
            nc.tensor.matmul(rec_ps[:], lhsT=ones_row[:], rhs=rec[:],
                             start=True, stop=True)
            rec_bc = smp.tile([128, 1], f32, tag="recbc")
            nc.scalar.copy(rec_bc[:], rec_ps[:])
            wn_bf = smp.tile([128, NCH], bf16, tag="wnbf")
            nc.scalar.activation(wn_bf[:], expT[:], AF.Copy, scale=rec_bc[:])
            wn_f = smp.tile([128, NCH], f32, tag="wnf")
            nc.scalar.activation(wn_f[:], expT[:], AF.Copy, scale=rec_bc[:])
            # weights out: transpose [128, 32] -> [32, 128], contiguous DMA
            wtr_ps = spp.tile([NCH, 128], f32, tag="sp")
            nc.tensor.transpose(wtr_ps[:], wn_f[:], id_sb[:])
            wtr_sb = smp.tile([NCH, 128], f32, tag="wtr")
            nc.vector.tensor_copy(wtr_sb[:], wtr_ps[:])
            nc.sync.dma_start(
                wts_out[b].rearrange("(c p) -> c p", p=128), wtr_sb[:]
            )
            return wn_bf

        def phase_c(b, wn_bf):
            """context = sum_n w[n] * img[b, n, :]."""
            cp = ctxps.tile([1, F], f32, tag="cp")
            for g in range(NCH // CG):
                inat = inp.tile([128, CG, F], bf16, tag="in")
                nc.sync.dma_start(inat[:], imgN[b, :, g * CG:(g + 1) * CG, :])
                for j in range(CG):
                    c = g * CG + j
                    nc.tensor.matmul(
                        cp[:],
                        lhsT=wn_bf[:, c:c + 1],
                        rhs=inat[:, j, :],
                        start=(c == 0),
                        stop=(c == NCH - 1),
                    )
            csb = csp.tile([1, F], f32, tag="csb")
            nc.vector.tensor_copy(csb[:], cp[:])
            nc.sync.dma_start(ctx_out[b:b + 1, :], csb[:])

        # software pipeline: A(0); then per b: B(b), A(b+1), C(b)
        phase_a(0)
        for b in range(BPC):
            wn_bf = phase_b(b)
            if b + 1 < BPC:
                phase_a(b + 1)
            phase_c(b, wn_bf)

    nc.compile()
    _nc_cache["nc"] = nc
    return nc


def _in_maps(image_features, hidden_state, W_img, W_hid, W_score):
    img_bf = image_features.astype(_BF)                        # [64, 4096, 512]
    # context rhs: [B, p, n-chunk, f] with f contiguous per (p, chunk)
    imgN_q = np.ascontiguousarray(
        img_bf.reshape(B, NCH, 128, F).transpose(0, 2, 1, 3)
    )                                                          # [64, 128, 32, 512]
    # proj rhs: [B, p, f-chunk, n] with n contiguous per (p, chunk)
    imgT_bf = img_bf.transpose(0, 2, 1)                        # [64, 512, 4096] view
    imgT_q = np.ascontiguousarray(
        imgT_bf.reshape(B, 4, 128, N).transpose(0, 2, 1, 3)
    )                                                          # [64, 128, 4, 4096]
    wimgT = np.ascontiguousarray(W_img.T).astype(_BF)          # [512, 256]
    whidT = W_hid.T.astype(np.float32)                         # [512, 256]
    wsc = np.ascontiguousarray(W_score.reshape(1, D).T).astype(_BF)  # [256, 1]
    eye = np.eye(128, dtype=np.float32)

    in_maps = []
    for c in range(NCORES):
        s = slice(c * BPC, (c + 1) * BPC)
        hwpack = np.concatenate(
            [whidT, hidden_state[s].T.astype(np.float32)], axis=1
        )  # [512, 264]
        in_maps.append({
            "imgT": imgT_q[s],
            "imgN": imgN_q[s],
            "hw": np.ascontiguousarray(hwpack),
            "wimgT": wimgT,
            "wscore": wsc,
            "ident": eye,
        })
    return in_maps


def kernel(image_features, hidden_state, W_img, W_hid, W_score):
    from concourse.bass_utils import run_bass_kernel_spmd

    nc = _build_nc()
    in_maps = _in_maps(image_features, hidden_state, W_img, W_hid, W_score)
    res = run_bass_kernel_spmd(nc, in_maps, list(range(NCORES))).results
    ctx = np.concatenate([r["ctx"] for r in res], axis=0)
    wts = np.concatenate([r["wts"] for r in res], axis=0)
    return (ctx, wts)
